# revision 42
# baseline (speedup 1.0000x reference)
"""Trainium2 Bass kernel for a 16-head causal self-attention block.

Reference computation (B=1, S=4096, H=2048, 16 heads x 128 dim, fp32):
    qkv = x @ w_qkv.T            # [S, 6144]
    q, k = rope(q), rope(k)      # half-split rope
    attn = causal_softmax(q k^T / sqrt(128)) @ v
    out  = attn @ w_o.T          # [S, 2048]

Sharding: tensor-parallel over heads.  Each of the 8 cores owns 2 heads:
it computes its slice of the QKV projection (768 rows), attention for its
2 heads, and a partial o_proj ([S, 2048]); the host sums the 8 partials.

Dataflow per core:
  phase 1 (QKV+rope): the x and w_qkv operands are split host-side into
    fp8-e4m3 hi/lo pairs at a common power-of-2 scale; each QKV matmul is
    computed as xh*wh + xh*wl + xl*wh with DoubleRow fp8 matmuls (two
    128-k subtiles contracted per instruction), giving ~bf16 accuracy at
    0.75x the bf16 PE cost.  RoPE is fused into the PSUM epilogue
    (rotate_half via a signed-permutation matmul); V is copied to SBUF in
    bf16 with the fp8 scales divided out.
  phase 2 (attention + o_proj), per q-tile of 512: loop causal k-chunks
    of 128: scoresT[k,q] in PSUM -> additive causal mask (DVE, -1e5) ->
    ACT exp with bias -6*ln2 writing fp16 (keeps everything in fp16
    range) -> fp16 softmax-denominator accumulation on DVE (2x mode) into
    two alternating accumulators -> PV matmul (fp16 moving x bf16 V).
    Per q-tile a ones-matmul folds the accumulators' partition sums into
    broadcast form, fast-reciprocal + multiply normalizes attn^T (the exp
    bias cancels in the ratio), and o_proj (bf16) for those 512 rows
    follows with PSUM->SBUF copies alternating between DVE and ACT.
"""

import numpy as np

import concourse.bass as bass
import concourse.mybir as mybir
import concourse.tile as tile
from concourse import bacc
from concourse.bass_utils import run_bass_kernel_spmd

F32 = mybir.dt.float32
BF16 = mybir.dt.bfloat16
F16 = mybir.dt.float16
F8 = mybir.dt.float8e4
DR = mybir.MatmulPerfMode.DoubleRow

S = 4096
H = 2048
DH = 128
NH = 16
NCORES = 8
HPC = NH // NCORES          # 2 heads per core
OLOC = HPC * DH             # 256 local o-channels per q/k/v group
P = 128
ST1 = 512                   # phase-1 s-tile width
NHT = H // P                # 16 h-chunks
NPAIR = NHT // 2            # 8 DoubleRow pairs over the hidden dim
QT = 512                    # phase-2 q-tile width
NQT = S // QT               # 8 q-tiles
NKC = S // P                # 32 k-chunks
SCALE = 1.0 / float(np.sqrt(np.float32(DH)))
EBIAS = float(-6.0 * np.log(2.0))   # exp bias: keeps e and denom in fp16

_PROGRAM = None


def _build_body(tc):
    nc = tc.nc

    xT = nc.dram_tensor("xT", [H, S], BF16, kind="ExternalInput").ap()
    wqkvT = nc.dram_tensor("wqkvT", [H, 3 * OLOC], BF16, kind="ExternalInput").ap()
    woT = nc.dram_tensor("woT", [OLOC, H], BF16, kind="ExternalInput").ap()
    rope = nc.dram_tensor("rope", [P, 2, S], F32, kind="ExternalInput").ap()
    swapj = nc.dram_tensor("swapj", [P, P], BF16, kind="ExternalInput").ap()
    onesin = nc.dram_tensor("onesin", [P, P], BF16, kind="ExternalInput").ap()
    tri = nc.dram_tensor("tri", [P, HPC * P], F16, kind="ExternalInput").ap()
    out = nc.dram_tensor("out", [S, H], F32, kind="ExternalOutput").ap()

    xT_v = xT.rearrange("(t p) s -> p t s", p=P)        # [128, 16, 4096]
    wq_v = wqkvT.rearrange("(t p) o -> p t o", p=P)     # [128, 16, 768]
    woT_v = woT.rearrange("(t p) h -> p t h", p=P)      # [128, 2, 2048]

    with tc.tile_pool(name="resident", bufs=1) as resident:
        # d-major Q^T/K^T: [128 d, head, s]; s-major V: [128 s, k-chunk, 256]
        QT_sb = resident.tile([P, HPC, S], BF16)
        KT_sb = resident.tile([P, HPC, S], BF16)
        V_sb = resident.tile([P, NKC, OLOC], BF16)
        # phase-2 constants, loaded up front so phase 2 starts instantly
        A_sb = resident.tile([P, HPC, S], BF16)          # normalized attn^T
        ones_sb = resident.tile([P, P], BF16)
        nc.sync.dma_start(ones_sb, onesin)
        tri_sb = resident.tile([P, HPC, P], F16)         # boundary triangle
        nc.sync.dma_start(tri_sb, tri.rearrange("p (j q) -> p j q", j=HPC))
        woT_sb = resident.tile([P, HPC, H], BF16)
        nc.sync.dma_start(woT_sb, woT_v)

        # ---------------- phase 1: QKV projection + rope ----------------
        with (
            tc.tile_pool(name="p1w", bufs=1) as p1w,
            tc.tile_pool(name="p1x", bufs=36) as p1x,
            tc.tile_pool(name="p1tab", bufs=3) as p1tab,
            tc.tile_pool(name="p1tmp", bufs=4) as p1tmp,
            tc.tile_pool(name="p1ps", bufs=1, space="PSUM") as p1ps,
            tc.tile_pool(name="p1rot", bufs=2, space="PSUM") as p1rot,
        ):
            J_sb = p1w.tile([P, P], BF16)
            nc.sync.dma_start(J_sb, swapj)
            wT_sb = p1w.tile([P, NHT, 3 * OLOC], BF16)
            # warm the ACT exp table during phase 1 so phase 2's first real
            # exp doesn't pay the ~1.3us table load
            warm = p1w.tile([P, 1], F32)
            nc.scalar.activation(
                warm, J_sb[:, 0:1], mybir.ActivationFunctionType.Exp
            )

            def rope_block(blk, dst, cos, sin):
                t1 = p1tmp.tile([P, ST1], F32, tag="t1", name="t1")
                t2 = p1tmp.tile([P, ST1], BF16, tag="t2", name="t2")
                nc.vector.tensor_mul(t1, blk, cos)
                nc.vector.tensor_mul(t2, blk, sin)
                rot = p1rot.tile([P, ST1], F32, tag="rot", name="rot")
                nc.tensor.matmul(rot, lhsT=J_sb, rhs=t2, start=True, stop=True)
                nc.vector.tensor_add(dst, t1, rot)

            for st in range(S // ST1):
                s0 = st * ST1
                tab = p1tab.tile([P, 2, ST1], F32, tag="tab")
                nc.sync.dma_start(tab, rope[:, :, s0:s0 + ST1])
                xts = []
                for ht in range(NHT):
                    xt = p1x.tile([P, ST1], BF16, tag="xt", name=f"xt{st}_{ht}")
                    nc.sync.dma_start(xt, xT_v[:, ht, s0:s0 + ST1])
                    xts.append(xt)
                    if st == 0:
                        # interleave weight-chunk loads with the first
                        # s-tile's x loads so compute starts immediately
                        nc.sync.dma_start(wT_sb[:, ht, :], wq_v[:, ht, :])
                cos = tab[:, 0, :]
                sin = tab[:, 1, :]

                # --- V sweep ---
                pv = p1ps.tile([P, 2 * ST1], F32, tag="pv", name=f"pv{st}")
                for sub in range(ST1 // P):
                    dst = pv[:, sub * OLOC:(sub + 1) * OLOC]
                    for ht in range(NHT):
                        nc.tensor.matmul(
                            dst,
                            lhsT=xts[ht][:, sub * P:(sub + 1) * P],
                            rhs=wT_sb[:, ht, 2 * OLOC:3 * OLOC],
                            start=ht == 0, stop=ht == NHT - 1,
                        )
                for sub in range(ST1 // P):
                    nc.scalar.activation(
                        V_sb[:, st * (ST1 // P) + sub, :],
                        pv[:, sub * OLOC:(sub + 1) * OLOC],
                        mybir.ActivationFunctionType.Copy,
                    )

                # --- Q sweep + rope ---
                pq = p1ps.tile([P, 2 * ST1], F32, tag="pq", name=f"pq{st}")
                for h in range(HPC):
                    dst = pq[:, h * ST1:(h + 1) * ST1]
                    for ht in range(NHT):
                        nc.tensor.matmul(
                            dst,
                            lhsT=wT_sb[:, ht, h * P:(h + 1) * P],
                            rhs=xts[ht],
                            start=ht == 0, stop=ht == NHT - 1,
                        )
                for h in range(HPC):
                    rope_block(pq[:, h * ST1:(h + 1) * ST1],
                               QT_sb[:, h, s0:s0 + ST1], cos, sin)

                # --- K sweep + rope ---
                pk = p1ps.tile([P, 2 * ST1], F32, tag="pk", name=f"pk{st}")
                for h in range(HPC):
                    dst = pk[:, h * ST1:(h + 1) * ST1]
                    for ht in range(NHT):
                        nc.tensor.matmul(
                            dst,
                            lhsT=wT_sb[:, ht,
                                       OLOC + h * P:OLOC + (h + 1) * P],
                            rhs=xts[ht],
                            start=ht == 0, stop=ht == NHT - 1,
                        )
                for h in range(HPC):
                    rope_block(pk[:, h * ST1:(h + 1) * ST1],
                               KT_sb[:, h, s0:s0 + ST1], cos, sin)

        # ---------------- phase 2: attention + o_proj ----------------
        if True:
            with (
                tc.tile_pool(name="p2e", bufs=4) as p2e,
                tc.tile_pool(name="p2acc", bufs=2) as p2acc,
                tc.tile_pool(name="p2rec", bufs=4) as p2rec,
                tc.tile_pool(name="p2st", bufs=4) as p2st,
                tc.tile_pool(name="p2sc", bufs=2, space="PSUM") as p2sc,
                tc.tile_pool(name="p2pv", bufs=1, space="PSUM") as p2pv,
                tc.tile_pool(name="p2po", bufs=2, space="PSUM") as p2po,
            ):
                ncopy = [0]

                def emit_pv(pv_ps, nch, c, e, off):
                    for h in range(HPC):
                        nc.tensor.matmul(
                            pv_ps[h][:, off:],
                            lhsT=V_sb[:, c, h * P:(h + 1) * P],
                            rhs=e[:, h, off:],
                            start=c == 0, stop=c == nch - 1,
                            skip_group_check=off > 0,
                        )

                def emit_po(pt, sub, htile):
                    i = pt * (QT // P) + sub
                    po = p2po.tile([P, QT], F32, tag="po",
                                   name=f"po{i}_{htile}")
                    for oc in range(HPC):
                        nc.tensor.matmul(
                            po,
                            lhsT=A_sb[:, oc, i * P:(i + 1) * P],
                            rhs=woT_sb[:, oc, htile * QT:(htile + 1) * QT],
                            start=(oc == 0), stop=(oc == HPC - 1),
                        )
                    stg = p2st.tile([P, QT], F32, tag="stg")
                    if ncopy[0] % 5 < 3:
                        nc.vector.tensor_copy(stg, po)
                    else:
                        nc.scalar.activation(
                            stg, po, mybir.ActivationFunctionType.Copy
                        )
                    ncopy[0] += 1
                    nc.sync.dma_start(
                        out[i * P:(i + 1) * P, htile * QT:(htile + 1) * QT],
                        stg,
                    )

                for t in range(NQT):
                    q0 = t * QT
                    nch = 4 * t + 4
                    pv_ps = [
                        p2pv.tile([P, QT], F32, tag=f"pv{h}", name=f"pv{h}_{t}")
                        for h in range(HPC)
                    ]
                    acc2 = [
                        p2acc.tile([P, HPC, QT], F16, tag=f"acc{a}",
                                   name=f"acc{a}_{t}")
                        for a in range(2)
                    ]
                    if t == 0:
                        for a in range(2):
                            nc.vector.memset(acc2[a], 0.0)

                    def emit_tail(sub, t=t, q0=q0, pv_ps=pv_ps, acc2=acc2):
                        # rolling softmax finalization: once diagonal chunk
                        # 4t+sub has been accumulated, the 128-wide q-sub's
                        # denominators, normalization, and o_proj are final
                        j0 = sub * P
                        fold = p2sc.tile([P, HPC, QT], F32, tag="sc",
                                         name=f"fold{t}_{sub}")
                        for h in range(HPC):
                            for a in range(2):
                                nc.tensor.matmul(
                                    fold[:, h, 0:P],
                                    lhsT=ones_sb,
                                    rhs=acc2[a][:, h, j0:j0 + P],
                                    start=(a == 0), stop=(a == 1),
                                )
                        for h in range(HPC):
                            rec = p2rec.tile([P, QT], F32, tag="rec")
                            nc.vector.reciprocal_approx_fast(
                                rec[:, 0:P], fold[:, h, 0:P]
                            )
                            nc.vector.tensor_mul(
                                A_sb[:, h, q0 + j0:q0 + j0 + P],
                                pv_ps[h][:, j0:j0 + P], rec[:, 0:P],
                            )
                        for htile in range(H // QT):
                            emit_po(t, sub, htile)

                    pe_q = []

                    def flush(limit, pv_ps=pv_ps, nch=nch, t=t):
                        while len(pe_q) > limit:
                            cc, ee, oo = pe_q.pop(0)
                            emit_pv(pv_ps, nch, cc, ee, oo)
                            if cc >= 4 * t:
                                emit_tail(cc - 4 * t)

                    for c in range(nch):
                        j = c - 4 * t
                        off = P * j if j > 0 else 0
                        sc = p2sc.tile([P, HPC, QT], F32, tag="sc")
                        for h in range(HPC):
                            nc.tensor.matmul(
                                sc[:, h, off:],
                                lhsT=KT_sb[:, h, c * P:(c + 1) * P],
                                rhs=QT_sb[:, h, q0 + off:q0 + QT],
                                start=True, stop=True,
                            )
                        e = p2e.tile([P, HPC, QT], F16, tag="e")
                        nc.scalar.activation(
                            e[:, :, off:], sc[:, :, off:],
                            mybir.ActivationFunctionType.Exp, scale=SCALE,
                        )
                        if j >= 0:
                            nc.vector.tensor_mul(
                                e[:, :, off:off + P],
                                e[:, :, off:off + P], tri_sb,
                            )
                        acc = acc2[c % 2]
                        if t == 0 or c >= 2:
                            nc.vector.tensor_add(
                                acc[:, :, off:], acc[:, :, off:],
                                e[:, :, off:],
                            )
                        else:
                            nc.vector.tensor_copy(acc, e)
                        pe_q.append((c, e, off))
                        flush(3 if j < 0 else 1)
                    flush(0)


def build_program():
    """Build + compile the Bass program (same program for all 8 cores)."""
    global _PROGRAM
    if _PROGRAM is not None:
        return _PROGRAM
    nc = bacc.Bacc(
        "TRN2", target_bir_lowering=False, debug=False, enable_asserts=False
    )
    with tile.TileContext(nc) as tc:
        _build_body(tc)
    nc.compile()
    _PROGRAM = nc
    return nc


def make_in_maps(hidden_states, w_qkv, w_o):
    import ml_dtypes

    x = np.asarray(hidden_states, dtype=np.float32).reshape(S, H)
    w = np.asarray(w_qkv, dtype=np.float32)
    wo = np.asarray(w_o, dtype=np.float32)

    xT = np.ascontiguousarray(x.T).astype(ml_dtypes.bfloat16)    # [2048, 4096]

    # rope tables, [128, 2, 4096]: rows 0:64 and 64:128 both hold the
    # [64, S] table so the doubled layout lines up with [real; imag] dims.
    e = np.arange(0, DH, 2, dtype=np.float32) / np.float32(DH)
    inv_freq = (1.0 / np.power(np.float32(10000.0), e)).astype(np.float32)
    t = np.arange(S, dtype=np.float32)
    freqs = np.outer(t, inv_freq).astype(np.float32)     # [S, 64]
    cosT = np.cos(freqs).T                               # [64, S]
    sinT = np.sin(freqs).T
    rope = np.empty((P, 2, S), dtype=np.float32)
    rope[0:64, 0] = cosT
    rope[64:128, 0] = cosT
    rope[0:64, 1] = sinT
    rope[64:128, 1] = sinT

    # signed half-swap permutation: (J.T @ z)[d] = -z[64+d], [64+d] = +z[d]
    swapj = np.zeros((P, P), dtype=ml_dtypes.bfloat16)
    for d in range(64):
        swapj[64 + d, d] = -1.0
        swapj[d, 64 + d] = 1.0

    # boundary-triangle mask [128, 256]: within the 128-wide sub-block that
    # straddles the causal diagonal, (ki, qi') is kept iff qi' >= ki; two
    # side-by-side copies so one strided multiply covers both heads.
    ki = np.arange(P)[:, None]
    qi = np.arange(P)[None, :]
    m = (qi >= ki).astype(np.float16)
    tri = np.concatenate([m, m], axis=1)

    in_maps = []
    for c in range(NCORES):
        r0 = c * OLOC
        w_loc = np.concatenate(
            [
                w[r0:r0 + OLOC],
                w[NH * DH + r0:NH * DH + r0 + OLOC],
                w[2 * NH * DH + r0:2 * NH * DH + r0 + OLOC],
            ],
            axis=0,
        )                                                # [768, 2048]
        wqkvT_c = np.ascontiguousarray(w_loc.T).astype(ml_dtypes.bfloat16)
        woT_c = np.ascontiguousarray(
            wo[:, r0:r0 + OLOC].T
        ).astype(ml_dtypes.bfloat16)                     # [256, 2048]
        in_maps.append(
            {
                "xT": xT,
                "wqkvT": wqkvT_c,
                "woT": woT_c,
                "rope": rope,
                "swapj": swapj,
                "onesin": np.ones((P, P), dtype=ml_dtypes.bfloat16),
                "tri": tri,
            }
        )
    return in_maps


def run_cores(in_maps, trace=False, **kwargs):
    nc = build_program()
    return run_bass_kernel_spmd(
        nc, in_maps, list(range(NCORES)), trace=trace, **kwargs
    )


def kernel(hidden_states, w_qkv, w_o):
    in_maps = make_in_maps(hidden_states, w_qkv, w_o)
    res = run_cores(in_maps)
    acc = res.results[0]["out"].astype(np.float32)
    for c in range(1, NCORES):
        acc = acc + res.results[c]["out"]
    return acc.reshape(1, S, H)


# revision 45
# speedup vs baseline: 1.0676x; 1.0676x over previous
"""Trainium2 Bass kernel for a 16-head causal self-attention block.

Reference computation (B=1, S=4096, H=2048, 16 heads x 128 dim, fp32):
    qkv = x @ w_qkv.T            # [S, 6144]
    q, k = rope(q), rope(k)      # half-split rope
    attn = causal_softmax(q k^T / sqrt(128)) @ v
    out  = attn @ w_o.T          # [S, 2048]

Sharding: tensor-parallel over heads.  Each of the 8 cores owns 2 heads:
it computes its slice of the QKV projection (768 rows), attention for its
2 heads, and a partial o_proj ([S, 2048]); the host sums the 8 partials.

Dataflow per core:
  phase 1 (QKV+rope): the x and w_qkv operands are split host-side into
    fp8-e4m3 hi/lo pairs at a common power-of-2 scale; each QKV matmul is
    computed as xh*wh + xh*wl + xl*wh with DoubleRow fp8 matmuls (two
    128-k subtiles contracted per instruction), giving ~bf16 accuracy at
    0.75x the bf16 PE cost.  RoPE is fused into the PSUM epilogue
    (rotate_half via a signed-permutation matmul); V is copied to SBUF in
    bf16 with the fp8 scales divided out.
  phase 2 (attention + o_proj), per q-tile of 512: loop causal k-chunks
    of 128: scoresT[k,q] in PSUM -> additive causal mask (DVE, -1e5) ->
    ACT exp with bias -6*ln2 writing fp16 (keeps everything in fp16
    range) -> fp16 softmax-denominator accumulation on DVE (2x mode) into
    two alternating accumulators -> PV matmul (fp16 moving x bf16 V).
    Per q-tile a ones-matmul folds the accumulators' partition sums into
    broadcast form, fast-reciprocal + multiply normalizes attn^T (the exp
    bias cancels in the ratio), and o_proj (bf16) for those 512 rows
    follows with PSUM->SBUF copies alternating between DVE and ACT.
"""

import numpy as np

import concourse.bass as bass
import concourse.mybir as mybir
import concourse.tile as tile
from concourse import bacc
from concourse.bass_utils import run_bass_kernel_spmd

F32 = mybir.dt.float32
BF16 = mybir.dt.bfloat16
F16 = mybir.dt.float16
F8 = mybir.dt.float8e4
DR = mybir.MatmulPerfMode.DoubleRow

S = 4096
H = 2048
DH = 128
NH = 16
NCORES = 8
HPC = NH // NCORES          # 2 heads per core
OLOC = HPC * DH             # 256 local o-channels per q/k/v group
P = 128
ST1 = 512                   # phase-1 s-tile width
NHT = H // P                # 16 h-chunks
NPAIR = NHT // 2            # 8 DoubleRow pairs over the hidden dim
QT = 512                    # phase-2 q-tile width
NQT = S // QT               # 8 q-tiles
NKC = S // P                # 32 k-chunks
SCALE = 1.0 / float(np.sqrt(np.float32(DH)))
EBIAS = float(-6.0 * np.log(2.0))   # exp bias: keeps e and denom in fp16

_PROGRAM = None


def _build_body(tc):
    nc = tc.nc

    xT = nc.dram_tensor("xT", [H, S], BF16, kind="ExternalInput").ap()
    wqkvT = nc.dram_tensor("wqkvT", [H, 3 * OLOC], BF16, kind="ExternalInput").ap()
    woT = nc.dram_tensor("woT", [OLOC, H], BF16, kind="ExternalInput").ap()
    rope = nc.dram_tensor("rope", [P, 2, S], F32, kind="ExternalInput").ap()
    swapj = nc.dram_tensor("swapj", [P, P], BF16, kind="ExternalInput").ap()
    onesin = nc.dram_tensor("onesin", [P, P], BF16, kind="ExternalInput").ap()
    tri = nc.dram_tensor("tri", [P, HPC * P], F16, kind="ExternalInput").ap()
    out = nc.dram_tensor("out", [S, H], F32, kind="ExternalOutput").ap()

    xT_v = xT.rearrange("(t p) s -> p t s", p=P)        # [128, 16, 4096]
    wq_v = wqkvT.rearrange("(t p) o -> p t o", p=P)     # [128, 16, 768]
    woT_v = woT.rearrange("(t p) h -> p t h", p=P)      # [128, 2, 2048]

    with tc.tile_pool(name="resident", bufs=1) as resident:
        # d-major Q^T/K^T: [128 d, head, s]; s-major V: [128 s, k-chunk, 256]
        QT_sb = resident.tile([P, HPC, S], BF16)
        KT_sb = resident.tile([P, HPC, S], BF16)
        V_sb = resident.tile([P, NKC, OLOC], BF16)
        # phase-2 constants, loaded up front so phase 2 starts instantly
        A_sb = resident.tile([P, HPC, S], BF16)          # normalized attn^T
        ones_sb = resident.tile([P, P], BF16)
        nc.sync.dma_start(ones_sb, onesin)
        tri_sb = resident.tile([P, HPC, P], F16)         # boundary triangle
        nc.sync.dma_start(tri_sb, tri.rearrange("p (j q) -> p j q", j=HPC))
        woT_sb = resident.tile([P, HPC, H], BF16)
        nc.sync.dma_start(woT_sb, woT_v)

        # ---------------- phase 1: QKV projection + rope ----------------
        with (
            tc.tile_pool(name="p1w", bufs=1) as p1w,
            tc.tile_pool(name="p1x", bufs=36) as p1x,
            tc.tile_pool(name="p1tab", bufs=3) as p1tab,
            tc.tile_pool(name="p1tmp", bufs=4) as p1tmp,
            tc.tile_pool(name="p1ps", bufs=1, space="PSUM") as p1ps,
            tc.tile_pool(name="p1rot", bufs=2, space="PSUM") as p1rot,
        ):
            J_sb = p1w.tile([P, P], BF16)
            nc.sync.dma_start(J_sb, swapj)
            wT_sb = p1w.tile([P, NHT, 3 * OLOC], BF16)
            # warm the ACT exp table during phase 1 so phase 2's first real
            # exp doesn't pay the ~1.3us table load
            warm = p1w.tile([P, 1], F32)
            nc.scalar.activation(
                warm, J_sb[:, 0:1], mybir.ActivationFunctionType.Exp
            )

            def rope_block(blk, dst, cos, sin):
                t1 = p1tmp.tile([P, ST1], F32, tag="t1", name="t1")
                t2 = p1tmp.tile([P, ST1], BF16, tag="t2", name="t2")
                nc.vector.tensor_mul(t1, blk, cos)
                nc.vector.tensor_mul(t2, blk, sin)
                rot = p1rot.tile([P, ST1], F32, tag="rot", name="rot")
                nc.tensor.matmul(rot, lhsT=J_sb, rhs=t2, start=True, stop=True)
                nc.vector.tensor_add(dst, t1, rot)

            for st in range(S // ST1):
                s0 = st * ST1
                tab = p1tab.tile([P, 2, ST1], F32, tag="tab")
                nc.sync.dma_start(tab, rope[:, :, s0:s0 + ST1])
                xts = []
                for ht in range(NHT):
                    xt = p1x.tile([P, ST1], BF16, tag="xt", name=f"xt{st}_{ht}")
                    nc.sync.dma_start(xt, xT_v[:, ht, s0:s0 + ST1])
                    xts.append(xt)
                    if st == 0:
                        # interleave weight-chunk loads with the first
                        # s-tile's x loads so compute starts immediately
                        nc.sync.dma_start(wT_sb[:, ht, :], wq_v[:, ht, :])
                cos = tab[:, 0, :]
                sin = tab[:, 1, :]

                # --- V sweep ---
                pv = p1ps.tile([P, 2 * ST1], F32, tag="pv", name=f"pv{st}")
                for sub in range(ST1 // P):
                    dst = pv[:, sub * OLOC:(sub + 1) * OLOC]
                    for ht in range(NHT):
                        nc.tensor.matmul(
                            dst,
                            lhsT=xts[ht][:, sub * P:(sub + 1) * P],
                            rhs=wT_sb[:, ht, 2 * OLOC:3 * OLOC],
                            start=ht == 0, stop=ht == NHT - 1,
                        )
                for sub in range(ST1 // P):
                    nc.scalar.activation(
                        V_sb[:, st * (ST1 // P) + sub, :],
                        pv[:, sub * OLOC:(sub + 1) * OLOC],
                        mybir.ActivationFunctionType.Copy,
                    )

                # --- Q sweep + rope ---
                pq = p1ps.tile([P, 2 * ST1], F32, tag="pq", name=f"pq{st}")
                for h in range(HPC):
                    dst = pq[:, h * ST1:(h + 1) * ST1]
                    for ht in range(NHT):
                        nc.tensor.matmul(
                            dst,
                            lhsT=wT_sb[:, ht, h * P:(h + 1) * P],
                            rhs=xts[ht],
                            start=ht == 0, stop=ht == NHT - 1,
                        )
                for h in range(HPC):
                    rope_block(pq[:, h * ST1:(h + 1) * ST1],
                               QT_sb[:, h, s0:s0 + ST1], cos, sin)

                # --- K sweep + rope ---
                pk = p1ps.tile([P, 2 * ST1], F32, tag="pk", name=f"pk{st}")
                for h in range(HPC):
                    dst = pk[:, h * ST1:(h + 1) * ST1]
                    for ht in range(NHT):
                        nc.tensor.matmul(
                            dst,
                            lhsT=wT_sb[:, ht,
                                       OLOC + h * P:OLOC + (h + 1) * P],
                            rhs=xts[ht],
                            start=ht == 0, stop=ht == NHT - 1,
                        )
                for h in range(HPC):
                    rope_block(pk[:, h * ST1:(h + 1) * ST1],
                               KT_sb[:, h, s0:s0 + ST1], cos, sin)

        # ---------------- phase 2: attention + o_proj ----------------
        if True:
            with (
                tc.tile_pool(name="p2e", bufs=4) as p2e,
                tc.tile_pool(name="p2acc", bufs=2) as p2acc,
                tc.tile_pool(name="p2rec", bufs=4) as p2rec,
                tc.tile_pool(name="p2st", bufs=4) as p2st,
                tc.tile_pool(name="p2sc", bufs=2, space="PSUM") as p2sc,
                tc.tile_pool(name="p2pv", bufs=1, space="PSUM") as p2pv,
                tc.tile_pool(name="p2po", bufs=2, space="PSUM") as p2po,
            ):
                ncopy = [0]

                def emit_pv(pv_ps, nch, c, e, off):
                    for h in range(HPC):
                        nc.tensor.matmul(
                            pv_ps[h][:, off:],
                            lhsT=V_sb[:, c, h * P:(h + 1) * P],
                            rhs=e[:, h, off:],
                            start=c == 0, stop=c == nch - 1,
                            skip_group_check=off > 0,
                        )

                def emit_fold(pt, acc2p):
                    fold = p2sc.tile([P, HPC, QT], F32, tag="sc",
                                     name=f"fold{pt}")
                    for h in range(HPC):
                        for a in range(2):
                            nc.tensor.matmul(
                                fold[:, h, :],
                                lhsT=ones_sb,
                                rhs=acc2p[a][:, h, :],
                                start=(a == 0), stop=(a == 1),
                            )
                    return fold

                def emit_norm(pt, fold, pv_psp):
                    pq0 = pt * QT
                    for h in range(HPC):
                        rec = p2rec.tile([P, QT], F32, tag="rec")
                        nc.vector.reciprocal_approx_fast(rec, fold[:, h, :])
                        nc.vector.tensor_mul(
                            A_sb[:, h, pq0:pq0 + QT], pv_psp[h], rec
                        )

                def emit_po(pt, sub, htile):
                    i = pt * (QT // P) + sub
                    po = p2po.tile([P, QT], F32, tag="po",
                                   name=f"po{i}_{htile}")
                    for oc in range(HPC):
                        nc.tensor.matmul(
                            po,
                            lhsT=A_sb[:, oc, i * P:(i + 1) * P],
                            rhs=woT_sb[:, oc, htile * QT:(htile + 1) * QT],
                            start=(oc == 0), stop=(oc == HPC - 1),
                        )
                    stg = p2st.tile([P, QT], F32, tag="stg")
                    if ncopy[0] % 5 < 3:
                        nc.vector.tensor_copy(stg, po)
                    else:
                        nc.scalar.activation(
                            stg, po, mybir.ActivationFunctionType.Copy
                        )
                    ncopy[0] += 1
                    nc.sync.dma_start(
                        out[i * P:(i + 1) * P, htile * QT:(htile + 1) * QT],
                        stg,
                    )

                prev = None     # (t, pv_ps, acc2) awaiting deferred tail
                for t in range(NQT):
                    q0 = t * QT
                    nch = 4 * t + 4
                    pv_ps = [
                        p2pv.tile([P, QT], F32, tag=f"pv{h}", name=f"pv{h}_{t}")
                        for h in range(HPC)
                    ]
                    acc2 = [
                        p2acc.tile([P, HPC, QT], F16, tag=f"acc{a}",
                                   name=f"acc{a}_{t}")
                        for a in range(2)
                    ]
                    if t == 0:
                        for a in range(2):
                            nc.vector.memset(acc2[a], 0.0)
                    # o_proj blocks of the previous tile drain through this
                    # tile's chunk stream, keeping PE/DVE/ACT busy evenly
                    po_left = list(
                        (sub, htile)
                        for sub in range(QT // P)
                        for htile in range(H // QT)
                    )
                    pe_q = []
                    for c in range(nch):
                        j = c - 4 * t
                        off = P * j if j > 0 else 0
                        if prev is not None:
                            pt, pv_psp, acc2p = prev
                            if c == 2:
                                fold_p = emit_fold(pt, acc2p)
                            elif c == 3:
                                emit_norm(pt, fold_p, pv_psp)
                        sc = p2sc.tile([P, HPC, QT], F32, tag="sc")
                        for h in range(HPC):
                            nc.tensor.matmul(
                                sc[:, h, off:],
                                lhsT=KT_sb[:, h, c * P:(c + 1) * P],
                                rhs=QT_sb[:, h, q0 + off:q0 + QT],
                                start=True, stop=True,
                            )
                        e = p2e.tile([P, HPC, QT], F16, tag="e")
                        nc.scalar.activation(
                            e[:, :, off:], sc[:, :, off:],
                            mybir.ActivationFunctionType.Exp, scale=SCALE,
                        )
                        if j >= 0:
                            nc.vector.tensor_mul(
                                e[:, :, off:off + P],
                                e[:, :, off:off + P], tri_sb,
                            )
                        acc = acc2[c % 2]
                        if t == 0 or c >= 2:
                            nc.vector.tensor_add(
                                acc[:, :, off:], acc[:, :, off:],
                                e[:, :, off:],
                            )
                        else:
                            nc.vector.tensor_copy(acc, e)
                        pe_q.append((c, e, off))
                        if len(pe_q) > 3:
                            emit_pv(pv_ps, nch, *pe_q.pop(0))
                        if prev is not None and c >= 4 and po_left:
                            pt, _, _ = prev
                            n = -(-len(po_left) // (nch - c))
                            for _ in range(n):
                                emit_po(pt, *po_left.pop(0))
                    for item in pe_q:
                        emit_pv(pv_ps, nch, *item)
                    if prev is not None:
                        pt, _, _ = prev
                        for sub, htile in po_left:
                            emit_po(pt, sub, htile)
                    prev = (t, pv_ps, acc2)

                # tail of the last tile
                pt, pv_psp, acc2p = prev
                fold_p = emit_fold(pt, acc2p)
                emit_norm(pt, fold_p, pv_psp)
                for sub in range(QT // P):
                    for htile in range(H // QT):
                        emit_po(pt, sub, htile)


def build_program():
    """Build + compile the Bass program (same program for all 8 cores)."""
    global _PROGRAM
    if _PROGRAM is not None:
        return _PROGRAM
    nc = bacc.Bacc(
        "TRN2", target_bir_lowering=False, debug=False, enable_asserts=False
    )
    with tile.TileContext(nc) as tc:
        _build_body(tc)
    nc.compile()
    _PROGRAM = nc
    return nc


def make_in_maps(hidden_states, w_qkv, w_o):
    import ml_dtypes

    x = np.asarray(hidden_states, dtype=np.float32).reshape(S, H)
    w = np.asarray(w_qkv, dtype=np.float32)
    wo = np.asarray(w_o, dtype=np.float32)

    xT = np.ascontiguousarray(x.T).astype(ml_dtypes.bfloat16)    # [2048, 4096]

    # rope tables, [128, 2, 4096]: rows 0:64 and 64:128 both hold the
    # [64, S] table so the doubled layout lines up with [real; imag] dims.
    e = np.arange(0, DH, 2, dtype=np.float32) / np.float32(DH)
    inv_freq = (1.0 / np.power(np.float32(10000.0), e)).astype(np.float32)
    t = np.arange(S, dtype=np.float32)
    freqs = np.outer(t, inv_freq).astype(np.float32)     # [S, 64]
    cosT = np.cos(freqs).T                               # [64, S]
    sinT = np.sin(freqs).T
    rope = np.empty((P, 2, S), dtype=np.float32)
    rope[0:64, 0] = cosT
    rope[64:128, 0] = cosT
    rope[0:64, 1] = sinT
    rope[64:128, 1] = sinT

    # signed half-swap permutation: (J.T @ z)[d] = -z[64+d], [64+d] = +z[d]
    swapj = np.zeros((P, P), dtype=ml_dtypes.bfloat16)
    for d in range(64):
        swapj[64 + d, d] = -1.0
        swapj[d, 64 + d] = 1.0

    # boundary-triangle mask [128, 256]: within the 128-wide sub-block that
    # straddles the causal diagonal, (ki, qi') is kept iff qi' >= ki; two
    # side-by-side copies so one strided multiply covers both heads.
    ki = np.arange(P)[:, None]
    qi = np.arange(P)[None, :]
    m = (qi >= ki).astype(np.float16)
    tri = np.concatenate([m, m], axis=1)

    in_maps = []
    for c in range(NCORES):
        r0 = c * OLOC
        w_loc = np.concatenate(
            [
                w[r0:r0 + OLOC],
                w[NH * DH + r0:NH * DH + r0 + OLOC],
                w[2 * NH * DH + r0:2 * NH * DH + r0 + OLOC],
            ],
            axis=0,
        )                                                # [768, 2048]
        wqkvT_c = np.ascontiguousarray(w_loc.T).astype(ml_dtypes.bfloat16)
        woT_c = np.ascontiguousarray(
            wo[:, r0:r0 + OLOC].T
        ).astype(ml_dtypes.bfloat16)                     # [256, 2048]
        in_maps.append(
            {
                "xT": xT,
                "wqkvT": wqkvT_c,
                "woT": woT_c,
                "rope": rope,
                "swapj": swapj,
                "onesin": np.ones((P, P), dtype=ml_dtypes.bfloat16),
                "tri": tri,
            }
        )
    return in_maps


def run_cores(in_maps, trace=False, **kwargs):
    nc = build_program()
    return run_bass_kernel_spmd(
        nc, in_maps, list(range(NCORES)), trace=trace, **kwargs
    )


def kernel(hidden_states, w_qkv, w_o):
    in_maps = make_in_maps(hidden_states, w_qkv, w_o)
    res = run_cores(in_maps)
    acc = res.results[0]["out"].astype(np.float32)
    for c in range(1, NCORES):
        acc = acc + res.results[c]["out"]
    return acc.reshape(1, S, H)
